# revision 1
# baseline (speedup 1.0000x reference)
"""BlockKoopmanNet forward on 8 Trainium2 NeuronCores (Bass/Tile).

Data-parallel over the batch: each core handles B/8 = 2048 rows.
Everything on-device is feature-major ([feature, batch] tiles) so every
layer is a plain lhsT(=weights).T @ rhs(=activations) matmul with no
on-device transposes.  Matmuls run as float32r (TF32-like, 1 cyc/row).

Host-side preprocessing folds all the awkward structure away:
  - x / u are fed pre-transposed; x is fed twice along the partition dim
    so the K=64 input layers run as two row-packed concurrent matmuls.
  - u is fed pre-tiled 8x along features for the Bu inner product.
  - The A(x) 2x2 rotation-scale uses column-broadcast copies of the
    a_w3/e_w3 heads so exp/cos/sin and the pair shuffle become pure
    per-partition ops: cos/sin/-sin are two Sin activations with
    per-partition phase biases (pi/2 shifts), DT is folded into scales.
  - Bu = einsum('bzu,bu->bz', ...) becomes an elementwise multiply with
    the tiled u followed by a 0/1 segment-sum matmul.
  - The output is produced transposed (yT) and un-transposed on host.
"""

import sys

sys.path.insert(0, "/opt/trn_rl_repo")

import numpy as np

DT = 0.02
B, X, U, Z, H, A = 16384, 64, 16, 32, 1024, 256
N_CORES = 8
BC = B // N_CORES  # 2048 rows per core
NB = 512  # batch tile width (matmul free dim)
NCHUNK = BC // NB  # 4

_CACHE = {}

# column offsets inside the packed small-weight tensor
OFF = {
    "e1": 0,       # 4 pairs x 128
    "a1": 512,
    "b1": 640,
    "a2": 768,     # 2 x 256
    "b2": 1280,
    "fpq": 1792,   # 2 x 64
    "b3": 1920,    # 2 x 512
    "z01": 2944,   # 8 x 64
    "seg": 3456,
    "d4": 3584,    # 8 x 64
    "d1": 4096,    # quad-packed: rows 32r..32r+32, col-group g, m = 4g+r
}
WCOLS = 4352
BCOLS = 64


def _build(loop=None):
    import concourse.bacc as bacc
    import concourse.mybir as mybir
    from concourse.tile import TileContext
    from contextlib import nullcontext

    F32 = mybir.dt.float32
    F32R = mybir.dt.float32r
    AF = mybir.ActivationFunctionType
    ALU = mybir.AluOpType

    nc = bacc.Bacc(
        "TRN2", target_bir_lowering=False, debug=False, num_devices=N_CORES
    )

    def din(name, shape, dt=F32R):
        return nc.dram_tensor(name, shape, dt, kind="ExternalInput").ap()

    x2T = din("x2T", (128, BC))
    uR = din("uR", (128, BC), F32)
    # all small weights packed into one per-partition-contiguous tensor
    wpack = din("wpack", (128, WCOLS))
    bpack = din("bpack", (128, BCOLS), F32)
    w_e2 = din("w_e2", (128, 8 * H))
    w_d2 = din("w_d2", (128, 8 * H))
    w_d3 = din("w_d3", (128, 8 * H))
    yT = nc.dram_tensor("yT", (X, BC), F32, kind="ExternalOutput").ap()

    with TileContext(nc) as tc:
        with (
            tc.tile_pool(name="wp", bufs=1) as wp,
            tc.tile_pool(name="hp", bufs=2) as hp,
            tc.tile_pool(name="abp", bufs=3) as abp,
            tc.tile_pool(name="xp", bufs=2) as xp,
            tc.tile_pool(name="up", bufs=2) as up,
            tc.tile_pool(name="prp", bufs=2) as prp,
            tc.tile_pool(name="mp", bufs=1) as mp,
            tc.tile_pool(name="znp", bufs=4) as znp,
            tc.tile_pool(name="yp", bufs=1) as yp,
            tc.tile_pool(name="pbig", bufs=6, space="PSUM") as pbig,
            tc.tile_pool(name="pmid", bufs=2, space="PSUM") as pmid,
        ):
            from concourse.tile_rust import add_dep_helper

            # one DMA for all small weights, one for all biases
            wpt = wp.tile([128, WCOLS], F32R, tag="wpt")
            bpt_t = wp.tile([128, BCOLS], F32, tag="bpt")

            def wload(ap, kc, m, tag, dma=nc.sync, dep=None):
                """Host-prearranged flat [128, kc*m] -> sbuf [128, kc, m]."""
                t = wp.tile([128, kc, m], F32R, tag=tag)
                inst = dma.dma_start(out=t[:].rearrange("p kc m -> p (kc m)"), in_=ap)
                if dep is not None:
                    add_dep_helper(inst.ins, dep.ins, reason="weight DMA ordering")
                return t

            # inputs for the first chunks + small weights go FIRST so the
            # input layers are not queued behind 12MB of big weights
            early_xu = {}
            for c in range(2):
                cs = c * NB
                ex = xp.tile([128, NB], F32R, tag="x")
                nc.sync.dma_start(out=ex, in_=x2T[:, cs : cs + NB])
                eu = up.tile([128, NB], F32, tag="u")
                nc.sync.dma_start(out=eu, in_=uR[:, cs : cs + NB])
                early_xu[c] = (ex, eu)

            WSPLIT = OFF["b3"]
            nc.sync.dma_start(out=wpt[:, :WSPLIT], in_=wpack[:, :WSPLIT])
            nc.sync.dma_start(out=bpt_t, in_=bpack)
            i_wp = nc.sync.dma_start(out=wpt[:, WSPLIT:], in_=wpack[:, WSPLIT:])
            bpt = bpt_t[:]

            # big weight matrices: idle gpsimd queue, held behind the small
            # pack (a gated DMA parks its whole issuing queue, so they must
            # not share a queue with compute-critical work)
            e2w = wp.tile([128, 8, H], F32R, tag="e2w")
            w_e2v = w_e2.rearrange("p (k m) -> p k m", k=8)
            i_e2a = nc.gpsimd.dma_start(out=e2w[:, :, : H // 2], in_=w_e2v[:, :, : H // 2])
            add_dep_helper(i_e2a.ins, i_wp.ins, reason="after small weights")
            i_e2b = nc.gpsimd.dma_start(out=e2w[:, :, H // 2 :], in_=w_e2v[:, :, H // 2 :])
            add_dep_helper(i_e2b.ins, i_wp.ins, reason="after small weights")
            d2w = wload(w_d2, 8, H, "d2w", dma=nc.gpsimd, dep=i_wp)
            d3w = wload(w_d3, 8, H, "d3w", dma=nc.gpsimd, dep=i_wp)

            wv = wpt[:]
            e1w = wv[:, OFF["e1"] : OFF["e1"] + 512]
            a1w = wv[:, OFF["a1"] : OFF["a1"] + 128]
            b1w = wv[:, OFF["b1"] : OFF["b1"] + 128]

            class PackedW:
                def __init__(self, name, M):
                    self.name, self.M = name, M

                def __getitem__(self, idx):
                    _, k, ms = idx
                    o = OFF[self.name] + k * self.M
                    lo = ms.start or 0
                    hi = self.M if ms.stop is None else ms.stop
                    return wv[:, o + lo : o + hi]

            a2w = PackedW("a2", A)
            b2w = PackedW("b2", A)
            b3w = PackedW("b3", Z * U)
            z01w = PackedW("z01", 2 * Z)
            fpqw = PackedW("fpq", 2 * Z)
            d4w = PackedW("d4", X)

            segt = wv[:, OFF["seg"] : OFF["seg"] + 128]
            d1w = wv[:, OFF["d1"] : OFF["d1"] + 256]

            e1b = bpt[:, 0:8]
            e2b = bpt[:, 8:16]
            a1b = bpt[:, 16:18]
            a2b = bpt[:, 18:20]
            b1b = bpt[:, 20:22]
            b2b = bpt[:, 22:24]
            b3b = bpt[:, 24:28]
            d1b = bpt[:, 28:36]
            d2b = bpt[:, 36:44]
            d3b = bpt[:, 44:52]
            z01b = bpt[:64, 52:53]
            fb = bpt[:32, 53:54]
            pb_hi = bpt[32:64, 54:55]
            qb_hi = bpt[32:64, 55:56]
            m0b = bpt[:32, 56:57]
            m1b = bpt[:32, 57:58]
            d4b = bpt[:64, 58:59]

            def mlp_layer(w_t, kc, b_t, rhs_fn, h_out, mtiles):
                """h_out[:, m, :] = silu(sum_k w.T @ rhs(k) + b) per m-chunk."""
                for mi in range(mtiles):
                    ps = pbig.tile([128, NB], F32, tag="pb")
                    for k in range(kc):
                        nc.tensor.matmul(
                            ps,
                            w_t[:, k, mi * 128 : (mi + 1) * 128],
                            rhs_fn(k),
                            start=(k == 0),
                            stop=(k == kc - 1),
                        )
                    nc.scalar.activation(
                        h_out[:, mi, :], ps, AF.Silu,
                        bias=b_t[:, mi : mi + 1], scale=1.0,
                    )

            def packed_pair(w_pair, x_t, b_t, h_out, j):
                """Two K=64 row-packed concurrent matmuls -> h m-chunks 2j, 2j+1."""
                psa = pbig.tile([128, NB], F32, tag="pb")
                psb = pbig.tile([128, NB], F32, tag="pb")
                nc.tensor.matmul(
                    psa, w_pair[0:64, :], x_t[0:64, :],
                    start=True, stop=True, tile_position=(0, 0),
                )
                nc.tensor.matmul(
                    psb, w_pair[64:128, :], x_t[64:128, :],
                    start=True, stop=True, tile_position=(64, 0),
                )
                nc.scalar.activation(
                    h_out[:, 2 * j, :], psa, AF.Silu,
                    bias=b_t[:, 2 * j : 2 * j + 1], scale=1.0,
                )
                nc.scalar.activation(
                    h_out[:, 2 * j + 1, :], psb, AF.Silu,
                    bias=b_t[:, 2 * j + 1 : 2 * j + 2], scale=1.0,
                )

            loop_ctx = tc.For_i(0, loop, 1) if loop is not None else nullcontext()
            with loop_ctx:
                zn_tiles = []
                # phase A: encoder + heads + latent step per chunk
                for c in range(NCHUNK):
                    cs = c * NB
                    if loop is None and c in early_xu:
                        x_t, u_t = early_xu[c]
                    else:
                        x_t = xp.tile([128, NB], F32R, tag="x")
                        nc.sync.dma_start(out=x_t, in_=x2T[:, cs : cs + NB])
                        u_t = up.tile([128, NB], F32, tag="u")
                        nc.sync.dma_start(out=u_t, in_=uR[:, cs : cs + NB])

                    # input layers + small heads first (only need x + small
                    # weights), so the e2w stream can still be in flight
                    h1 = hp.tile([128, 8, NB], F32R, tag="h")
                    for j in range(4):
                        packed_pair(e1w[:, j * 128 : (j + 1) * 128], x_t, e1b, h1, j)

                    # aux head (A(x) params)
                    ha1 = abp.tile([128, 2, NB], F32R, tag="ab")
                    packed_pair(a1w, x_t, a1b, ha1, 0)
                    ha2 = abp.tile([128, 2, NB], F32R, tag="ab")
                    mlp_layer(a2w, 2, a2b, lambda k: ha1[:, k, :], ha2, 2)

                    # B(x) head
                    hb1 = abp.tile([128, 2, NB], F32R, tag="ab")
                    packed_pair(b1w, x_t, b1b, hb1, 0)
                    hb2 = abp.tile([128, 2, NB], F32R, tag="ab")
                    mlp_layer(b2w, 2, b2b, lambda k: hb1[:, k, :], hb2, 2)

                    # big encoder layer, with the F|P|Q head block slotted
                    # in after two m-chunks: the Exp/Sin table loads then
                    # happen while PE grinds e2, not at the chunk boundary
                    h2 = hp.tile([128, 8, NB], F32R, tag="h")
                    mlp_layer(e2w, 8, e2b, lambda k: h1[:, k, :], h2, 2)

                    # F | P | Q heads: psum [64, NB]; rows 0-31 drive F,
                    # rows 32-63 drive both P and Q (phase-shifted sins)
                    pfpq = pmid.tile([2 * Z, NB], F32, tag="pm")
                    for k in range(2):
                        nc.tensor.matmul(
                            pfpq, fpqw[:, k, :], ha2[:, k, :],
                            start=(k == 0), stop=(k == 1),
                        )
                    f_t = mp.tile([Z, NB], F32, tag="F")
                    nc.scalar.activation(f_t, pfpq[:Z], AF.Exp, bias=fb, scale=DT)
                    p_t = mp.tile([Z, NB], F32, tag="P")
                    nc.scalar.activation(p_t, pfpq[Z:], AF.Sin, bias=pb_hi, scale=DT)
                    q_t = mp.tile([Z, NB], F32, tag="Q")
                    nc.scalar.activation(q_t, pfpq[Z:], AF.Sin, bias=qb_hi, scale=DT)

                    for mi in range(2, 8):
                        ps = pbig.tile([128, NB], F32, tag="pb")
                        for k in range(8):
                            nc.tensor.matmul(
                                ps, e2w[:, k, mi * 128 : (mi + 1) * 128],
                                h1[:, k, :],
                                start=(k == 0), stop=(k == 7),
                            )
                        nc.scalar.activation(
                            h2[:, mi, :], ps, AF.Silu,
                            bias=e2b[:, mi : mi + 1], scale=1.0,
                        )

                    # z pair-broadcasts Z0|Z1 in one [64, NB] psum
                    pz = pmid.tile([2 * Z, NB], F32, tag="pm")
                    for k in range(8):
                        nc.tensor.matmul(
                            pz, z01w[:, k, :], h2[:, k, :],
                            start=(k == 0), stop=(k == 7),
                        )
                    z0_t = mp.tile([Z, NB], F32, tag="Z0")
                    nc.vector.tensor_scalar_add(
                        out=z0_t[:], in0=pz[:Z], scalar1=z01b[:Z, 0:1]
                    )
                    z1_t = mp.tile([Z, NB], F32, tag="Z1")
                    nc.vector.tensor_scalar_add(
                        out=z1_t[:], in0=pz[Z:], scalar1=z01b[Z:, 0:1]
                    )

                    # Bflat + Bu
                    prods = []
                    for mc in range(4):
                        psb = pbig.tile([128, NB], F32, tag="pb")
                        for k in range(2):
                            nc.tensor.matmul(
                                psb, b3w[:, k, mc * 128 : (mc + 1) * 128],
                                hb2[:, k, :],
                                start=(k == 0), stop=(k == 1),
                            )
                        pr = prp.tile([128, NB], F32R, tag="prod")
                        nc.vector.scalar_tensor_tensor(
                            out=pr[:], in0=psb[:], scalar=b3b[:, mc : mc + 1],
                            in1=u_t[:], op0=ALU.add, op1=ALU.mult,
                        )
                        prods.append(pr)
                    pbu = pmid.tile([Z, NB], F32, tag="pm")
                    for mc in range(4):
                        nc.tensor.matmul(
                            pbu, segt[:, mc * 32 : (mc + 1) * 32], prods[mc],
                            start=(mc == 0), stop=(mc == 3),
                        )

                    # z_next = G0*Z0 + G1*Z1 + DT*Bu   (in-place DVE chain)
                    nc.vector.tensor_tensor(
                        out=p_t[:], in0=f_t[:], in1=p_t[:], op=ALU.mult
                    )
                    nc.vector.tensor_scalar(
                        out=p_t[:], in0=p_t[:], scalar1=DT, scalar2=m0b,
                        op0=ALU.mult, op1=ALU.add,
                    )
                    nc.vector.tensor_tensor(
                        out=q_t[:], in0=f_t[:], in1=q_t[:], op=ALU.mult
                    )
                    nc.vector.tensor_scalar(
                        out=q_t[:], in0=q_t[:], scalar1=DT, scalar2=m1b,
                        op0=ALU.mult, op1=ALU.add,
                    )
                    nc.vector.tensor_tensor(
                        out=p_t[:], in0=p_t[:], in1=z0_t[:], op=ALU.mult
                    )
                    nc.vector.tensor_tensor(
                        out=q_t[:], in0=q_t[:], in1=z1_t[:], op=ALU.mult
                    )
                    nc.vector.tensor_tensor(
                        out=p_t[:], in0=p_t[:], in1=q_t[:], op=ALU.add
                    )
                    zn_t = znp.tile([128, NB], F32R, tag="zn")
                    nc.vector.scalar_tensor_tensor(
                        out=zn_t[:Z], in0=pbu[:], scalar=DT, in1=p_t[:],
                        op0=ALU.mult, op1=ALU.add,
                    )
                    # replicate to all 4 row-groups for quad-packed d1
                    nc.vector.tensor_copy(out=zn_t[Z : 2 * Z], in_=zn_t[:Z])
                    nc.vector.tensor_copy(out=zn_t[2 * Z :], in_=zn_t[: 2 * Z])
                    zn_tiles.append(zn_t)

                # phase B: decoders, pipelined behind phase A on PE
                for c in range(NCHUNK):
                    cs = c * NB
                    zn_t = zn_tiles[c]
                    hd1 = hp.tile([128, 8, NB], F32R, tag="h")
                    for g in range(2):
                        pss = [pbig.tile([128, NB], F32, tag="pb", name=f"d1ps{_r}") for _r in range(4)]
                        for r in range(4):
                            nc.tensor.matmul(
                                pss[r],
                                d1w[32 * r : 32 * r + 32, g * 128 : (g + 1) * 128],
                                zn_t[32 * r : 32 * r + 32, :],
                                start=True, stop=True,
                                tile_position=(32 * r, 0),
                            )
                        for r in range(4):
                            mi = 4 * g + r
                            nc.scalar.activation(
                                hd1[:, mi, :], pss[r], AF.Silu,
                                bias=d1b[:, mi : mi + 1], scale=1.0,
                            )
                    hd2 = hp.tile([128, 8, NB], F32R, tag="h")
                    mlp_layer(d2w, 8, d2b, lambda k: hd1[:, k, :], hd2, 8)
                    hd3 = hp.tile([128, 8, NB], F32R, tag="h")
                    mlp_layer(d3w, 8, d3b, lambda k: hd2[:, k, :], hd3, 8)

                    py_full = pbig.tile([128, NB], F32, tag="pb")
                    py_t = py_full[:X]
                    for k in range(8):
                        nc.tensor.matmul(
                            py_t, d4w[:, k, :], hd3[:, k, :],
                            start=(k == 0), stop=(k == 7),
                        )
                    y_sb = yp.tile([X, NB], F32, tag="y")
                    nc.vector.tensor_scalar_add(
                        out=y_sb[:], in0=py_t[:], scalar1=d4b
                    )
                    nc.sync.dma_start(out=yT[:, cs : cs + NB], in_=y_sb)

    nc.compile()
    return nc


def _prep_host(inputs):
    f32 = np.float32
    x = np.asarray(inputs["x"], f32)
    u = np.asarray(inputs["u"], f32)

    xT = np.ascontiguousarray(x.T)
    x2T = np.concatenate([xT, xT], axis=0)  # [128, B]: x twice (row packing)
    uR = np.tile(np.ascontiguousarray(u.T), (8, 1))  # [128, B]

    def fm(w):
        """[K, M] -> [128, (K//128)*M]: per-partition-contiguous lhsT chunks."""
        kc = w.shape[0] // 128
        return np.ascontiguousarray(
            w.reshape(kc, 128, w.shape[1]).transpose(1, 0, 2).reshape(128, -1)
        )

    def pack_pairs(w):
        """[64, M] -> [128, M//... ] row-packed pairs of 128-col chunks."""
        mt = w.shape[1] // 256
        out = np.zeros((128, mt, 128), f32)
        for j in range(mt):
            out[:64, j] = w[:, (2 * j) * 128 : (2 * j + 1) * 128]
            out[64:, j] = w[:, (2 * j + 1) * 128 : (2 * j + 2) * 128]
        return out

    idx0 = np.arange(Z) // 2 * 2
    idx1 = idx0 + 1
    even = (np.arange(Z) % 2 == 0).astype(f32)

    e_w3 = np.asarray(inputs["e_w3"], f32)
    e_b3 = np.asarray(inputs["e_b3"], f32)
    a_w3 = np.asarray(inputs["a_w3"], f32)
    a_b3 = np.asarray(inputs["a_b3"], f32)

    segw = np.zeros((128, 128), f32)
    for mc in range(4):
        for k in range(128):
            segw[k, mc * 32 + 8 * mc + k // 16] = 1.0

    pi = np.pi

    wpack = np.zeros((128, WCOLS), f32)
    wpack[:, OFF["e1"] : OFF["e1"] + 512] = pack_pairs(
        np.asarray(inputs["e_w1"], f32)
    ).reshape(128, 512)
    wpack[:, OFF["a1"] : OFF["a1"] + 128] = pack_pairs(
        np.asarray(inputs["a_w1"], f32)
    )[:, 0]
    wpack[:, OFF["b1"] : OFF["b1"] + 128] = pack_pairs(
        np.asarray(inputs["b_w1"], f32)
    )[:, 0]
    wpack[:, OFF["a2"] : OFF["a2"] + 512] = fm(np.asarray(inputs["a_w2"], f32))
    wpack[:, OFF["b2"] : OFF["b2"] + 512] = fm(np.asarray(inputs["b_w2"], f32))
    wpack[:, OFF["fpq"] : OFF["fpq"] + 128] = fm(
        np.concatenate([a_w3[:, idx0], a_w3[:, idx1]], axis=1)
    )
    wpack[:, OFF["b3"] : OFF["b3"] + 1024] = fm(np.asarray(inputs["b_w3"], f32))
    wpack[:, OFF["z01"] : OFF["z01"] + 512] = fm(
        np.concatenate([e_w3[:, idx0], e_w3[:, idx1]], axis=1)
    )
    wpack[:, OFF["seg"] : OFF["seg"] + 128] = segw
    wpack[:, OFF["d4"] : OFF["d4"] + 512] = fm(np.asarray(inputs["d_w4"], f32))
    d_w1 = np.asarray(inputs["d_w1"], f32)
    for g in range(2):
        for r in range(4):
            m = 4 * g + r
            wpack[32 * r : 32 * r + 32, OFF["d1"] + g * 128 : OFF["d1"] + (g + 1) * 128] = (
                d_w1[:, m * 128 : (m + 1) * 128]
            )

    def bcol(b):
        return np.asarray(b, f32).reshape(-1, 128).T

    bpack = np.zeros((128, BCOLS), f32)
    bpack[:, 0:8] = bcol(inputs["e_b1"])
    bpack[:, 8:16] = bcol(inputs["e_b2"])
    bpack[:, 16:18] = bcol(inputs["a_b1"])
    bpack[:, 18:20] = bcol(inputs["a_b2"])
    bpack[:, 20:22] = bcol(inputs["b_b1"])
    bpack[:, 22:24] = bcol(inputs["b_b2"])
    bpack[:, 24:28] = bcol(inputs["b_b3"])
    bpack[:, 28:36] = bcol(inputs["d_b1"])
    bpack[:, 36:44] = bcol(inputs["d_b2"])
    bpack[:, 44:52] = bcol(inputs["d_b3"])
    bpack[:64, 52] = np.concatenate([e_b3[idx0], e_b3[idx1]])
    bpack[:32, 53] = DT * a_b3[idx0]
    bpack[32:64, 54] = DT * a_b3[idx1] + even * (pi / 2)
    bpack[32:64, 55] = DT * a_b3[idx1] + np.where(even, pi, pi / 2)
    bpack[:32, 56] = even
    bpack[:32, 57] = 1.0 - even
    bpack[:64, 58] = np.asarray(inputs["d_b4"], f32)

    shared = {
        "wpack": wpack,
        "bpack": bpack,
        "w_e2": fm(np.asarray(inputs["e_w2"], f32)),
        "w_d2": fm(np.asarray(inputs["d_w2"], f32)),
        "w_d3": fm(np.asarray(inputs["d_w3"], f32)),
    }

    in_maps = []
    for c in range(N_CORES):
        sl = slice(c * BC, (c + 1) * BC)
        m = dict(shared)
        m["x2T"] = np.ascontiguousarray(x2T[:, sl])
        m["uR"] = np.ascontiguousarray(uR[:, sl])
        in_maps.append(m)
    return in_maps


def kernel(**inputs) -> np.ndarray:
    from concourse import bass_utils

    if "nc" not in _CACHE:
        _CACHE["nc"] = _build()
    nc = _CACHE["nc"]
    in_maps = _prep_host(inputs)
    res = bass_utils.run_bass_kernel_spmd(
        nc, in_maps, core_ids=list(range(N_CORES))
    )
    return np.concatenate(
        [np.asarray(res.results[c]["yT"]).T for c in range(N_CORES)], axis=0
    ).astype(np.float32)



# revision 2
# speedup vs baseline: 329.1187x; 329.1187x over previous
"""BlockKoopmanNet forward on 8 Trainium2 NeuronCores (Bass/Tile).

Data-parallel over the batch: each core handles B/8 = 2048 rows.
Everything on-device is feature-major ([feature, batch] tiles) so every
layer is a plain lhsT(=weights).T @ rhs(=activations) matmul with no
on-device transposes.

Matmul precision strategy:
  - K>=256 layers (e2, d2, d3, a2, b2, b3, z01-head, Bu segment-sum) run
    as float8e4 (e4m3) matmuls in DoubleRow perf mode: each instruction
    contracts TWO 128-row k-groups at once (lhsT [128,2,M], rhs
    [128,2,N]).  Weights are pre-scaled by 2^10 on host (e4m3 sweet
    spot); the 2^-10 descale folds into the following activation's
    `scale` (or the u/bias prep for b3).  Verified against the reference
    on CPU: rel_absmax ~1e-2 < 2e-2 gate.
  - K<=64 layers (e1, a1, b1, fpq-head, d1, d4) stay float32r with the
    row-packed / quad-packed tile_position tricks.
  - exp(a*DT) is computed as tanh: u=tanh(a*DT/2); f=(1+u)/(1-u) via DVE
    reciprocal, so Silu/Sin/Tanh all live in one ACT table set
    (`silu_and_others`) and the per-chunk activation-table reloads
    disappear.

Host-side preprocessing folds all the awkward structure away:
  - x / u are fed pre-transposed; x is fed twice along the partition dim
    so the K=64 input layers run as two row-packed concurrent matmuls.
  - u is fed pre-tiled 8x along features for the Bu inner product and
    pre-scaled by PR_SCALE/S8 (fp8 b3 descale + product-tile ranging).
  - The A(x) 2x2 rotation-scale uses column-broadcast copies of the
    a_w3/e_w3 heads so tanh/cos/sin and the pair shuffle become pure
    per-partition ops: cos/sin/-sin are two Sin activations with
    per-partition phase biases (pi/2 shifts), DT is folded into scales.
  - Bu = einsum('bzu,bu->bz', ...) becomes an elementwise multiply with
    the tiled u followed by a 0/1 segment-sum DoubleRow matmul.
  - The output is produced transposed (yT) and un-transposed on host.
"""

import sys

sys.path.insert(0, "/opt/trn_rl_repo")

import numpy as np

DT = 0.02
B, X, U, Z, H, A = 16384, 64, 16, 32, 1024, 256
N_CORES = 8
BC = B // N_CORES  # 2048 rows per core
NB = 512  # batch tile width (matmul free dim)
NCHUNK = BC // NB  # 4

S8 = 1024.0   # fp8 weight pre-scale (2^10)
INV8 = 1.0 / S8
PR_SCALE = 16.0  # extra ranging for the fp8 product tiles

_CACHE = {}

# column offsets inside the packed float32r small-weight tensor
OFF = {
    "e1": 0,      # 4 pairs x 128
    "a1": 512,
    "b1": 640,
    "fpq": 768,   # 2 x 64
    "d1": 896,    # quad-packed: rows 32r..32r+32, col-group g, m = 4g+r
    "d4": 1152,   # 8 x 64
}
WCOLS = 1664
# per-k-group column offsets inside the packed fp8 tensor [128, 2, W8COLS]
OFF8 = {
    "a2": 0,      # 2 x 128
    "b2": 256,
    "b3": 512,    # 4 x 128
    "seg": 1024,  # 2 pairs x 32
    "z01": 1088,  # 4 pairs x 64
}
W8COLS = 1344
BCOLS = 64


def _build(loop=None):
    import concourse.bacc as bacc
    import concourse.mybir as mybir
    from concourse.tile import TileContext
    from contextlib import nullcontext

    F32 = mybir.dt.float32
    F32R = mybir.dt.float32r
    F8 = mybir.dt.float8e4
    AF = mybir.ActivationFunctionType
    ALU = mybir.AluOpType
    DR = mybir.MatmulPerfMode.DoubleRow

    nc = bacc.Bacc(
        "TRN2", target_bir_lowering=False, debug=False, num_devices=N_CORES
    )

    def din(name, shape, dt=F32R):
        return nc.dram_tensor(name, shape, dt, kind="ExternalInput").ap()

    x2T = din("x2T", (128, BC))
    uR = din("uR", (128, BC), F32)
    wpack = din("wpack", (128, WCOLS))
    wpack8 = din("wpack8", (128, 2 * W8COLS), F8)
    bpack = din("bpack", (128, BCOLS), F32)
    w_e2 = din("w_e2", (128, 8 * H), F8)
    w_d2 = din("w_d2", (128, 8 * H), F8)
    w_d3 = din("w_d3", (128, 8 * H), F8)
    yT = nc.dram_tensor("yT", (X, BC), F32, kind="ExternalOutput").ap()

    with TileContext(nc) as tc:
        with (
            tc.tile_pool(name="wp", bufs=1) as wp,
            tc.tile_pool(name="hp", bufs=2) as hp,
            tc.tile_pool(name="abp", bufs=3) as abp,
            tc.tile_pool(name="xp", bufs=2) as xp,
            tc.tile_pool(name="up", bufs=2) as up,
            tc.tile_pool(name="prp", bufs=2) as prp,
            tc.tile_pool(name="mp", bufs=2) as mp,
            tc.tile_pool(name="znp", bufs=4) as znp,
            tc.tile_pool(name="yp", bufs=1) as yp,
            tc.tile_pool(name="pbig", bufs=6, space="PSUM") as pbig,
            tc.tile_pool(name="pmid", bufs=2, space="PSUM") as pmid,
        ):
            from concourse.tile_rust import add_dep_helper

            wpt = wp.tile([128, WCOLS], F32R, tag="wpt")
            w8t = wp.tile([128, 2, W8COLS], F8, tag="w8t")
            bpt_t = wp.tile([128, BCOLS], F32, tag="bpt")

            def wload(ap, kc, m, tag, dma=nc.sync, dep=None, dt=F32R):
                """Host-prearranged flat [128, kc*m] -> sbuf [128, kc, m]."""
                t = wp.tile([128, kc, m], dt, tag=tag)
                inst = dma.dma_start(out=t[:].rearrange("p kc m -> p (kc m)"), in_=ap)
                if dep is not None:
                    add_dep_helper(inst.ins, dep.ins, reason="weight DMA ordering")
                return t

            # inputs for the first chunks go FIRST so the input layers are
            # not queued behind the big weights
            early_xu = {}
            for c in range(2):
                cs = c * NB
                ex = xp.tile([128, NB], F32R, tag="x")
                nc.sync.dma_start(out=ex, in_=x2T[:, cs : cs + NB])
                eu = up.tile([128, NB], F32, tag="u")
                nc.sync.dma_start(out=eu, in_=uR[:, cs : cs + NB])
                early_xu[c] = (ex, eu)

            WSPLIT = OFF["d1"]
            nc.sync.dma_start(out=wpt[:, :WSPLIT], in_=wpack[:, :WSPLIT])
            nc.sync.dma_start(out=bpt_t, in_=bpack)
            i_w8 = nc.sync.dma_start(
                out=w8t[:].rearrange("p i m -> p (i m)"), in_=wpack8
            )
            i_wp = nc.sync.dma_start(out=wpt[:, WSPLIT:], in_=wpack[:, WSPLIT:])
            bpt = bpt_t[:]

            # big weight matrices on the idle gpsimd queue, held behind the
            # small packs (a gated DMA parks its whole issuing queue)
            e2w = wload(w_e2, 8, H, "e2w", dma=nc.gpsimd, dep=i_w8, dt=F8)
            d2w = wload(w_d2, 8, H, "d2w", dma=nc.gpsimd, dep=i_wp, dt=F8)
            d3w = wload(w_d3, 8, H, "d3w", dma=nc.gpsimd, dep=i_wp, dt=F8)

            wv = wpt[:]
            e1w = wv[:, OFF["e1"] : OFF["e1"] + 512]
            a1w = wv[:, OFF["a1"] : OFF["a1"] + 128]
            b1w = wv[:, OFF["b1"] : OFF["b1"] + 128]
            d1w = wv[:, OFF["d1"] : OFF["d1"] + 256]

            class PackedW:
                def __init__(self, name, M):
                    self.name, self.M = name, M

                def __getitem__(self, idx):
                    _, k, ms = idx
                    o = OFF[self.name] + k * self.M
                    lo = ms.start or 0
                    hi = self.M if ms.stop is None else ms.stop
                    return wv[:, o + lo : o + hi]

            fpqw = PackedW("fpq", 2 * Z)
            d4w = PackedW("d4", X)

            def w8(name, lo, hi):
                o = OFF8[name]
                return w8t[:, :, o + lo : o + hi]

            e1b = bpt[:, 0:8]
            e2b = bpt[:, 8:16]
            a1b = bpt[:, 16:18]
            a2b = bpt[:, 18:20]
            b1b = bpt[:, 20:22]
            b2b = bpt[:, 22:24]
            b3b = bpt[:, 24:28]
            d1b = bpt[:, 28:36]
            d2b = bpt[:, 36:44]
            d3b = bpt[:, 44:52]
            z01b = bpt[:64, 52:53]
            fb = bpt[:32, 53:54]
            pb_hi = bpt[32:64, 54:55]
            qb_hi = bpt[32:64, 55:56]
            m0b = bpt[:32, 56:57]
            m1b = bpt[:32, 57:58]
            d4b = bpt[:64, 58:59]

            def mlp_layer(w_t, kc, b_t, rhs_fn, h_out, mtiles):
                """fp32r: h_out[:, m, :] = silu(sum_k w.T @ rhs(k) + b)."""
                for mi in range(mtiles):
                    ps = pbig.tile([128, NB], F32, tag="pb")
                    for k in range(kc):
                        nc.tensor.matmul(
                            ps,
                            w_t[:, k, mi * 128 : (mi + 1) * 128],
                            rhs_fn(k),
                            start=(k == 0),
                            stop=(k == kc - 1),
                        )
                    nc.scalar.activation(
                        h_out[:, mi, :], ps, AF.Silu,
                        bias=b_t[:, mi : mi + 1], scale=1.0,
                    )

            def dr_layer(w_fn, kpairs, b_t, rhs_t, h_out, mtiles):
                """fp8 DoubleRow: h_out[:,m,:] = silu(acc/S8 + b)."""
                for mi in range(mtiles):
                    ps = pbig.tile([128, NB], F32, tag="pb")
                    for g in range(kpairs):
                        nc.tensor.matmul(
                            ps, w_fn(g, mi), rhs_t[:, 2 * g : 2 * g + 2, :],
                            start=(g == 0), stop=(g == kpairs - 1),
                            perf_mode=DR,
                        )
                    nc.scalar.activation(
                        h_out[:, mi, :], ps, AF.Silu,
                        bias=b_t[:, mi : mi + 1], scale=INV8,
                    )

            def packed_pair(w_pair, x_t, b_t, h_out, j):
                """Two K=64 row-packed matmuls -> h m-chunks 2j, 2j+1."""
                psa = pbig.tile([128, NB], F32, tag="pb")
                psb = pbig.tile([128, NB], F32, tag="pb")
                nc.tensor.matmul(
                    psa, w_pair[0:64, :], x_t[0:64, :],
                    start=True, stop=True, tile_position=(0, 0),
                )
                nc.tensor.matmul(
                    psb, w_pair[64:128, :], x_t[64:128, :],
                    start=True, stop=True, tile_position=(64, 0),
                )
                nc.scalar.activation(
                    h_out[:, 2 * j, :], psa, AF.Silu,
                    bias=b_t[:, 2 * j : 2 * j + 1], scale=1.0,
                )
                nc.scalar.activation(
                    h_out[:, 2 * j + 1, :], psb, AF.Silu,
                    bias=b_t[:, 2 * j + 1 : 2 * j + 2], scale=1.0,
                )

            loop_ctx = tc.For_i(0, loop, 1) if loop is not None else nullcontext()
            with loop_ctx:
                zn_tiles = []
                # phase A: encoder + heads + latent step per chunk
                for c in range(NCHUNK):
                    cs = c * NB
                    if loop is None and c in early_xu:
                        x_t, u_t = early_xu[c]
                    else:
                        x_t = xp.tile([128, NB], F32R, tag="x")
                        nc.sync.dma_start(out=x_t, in_=x2T[:, cs : cs + NB])
                        u_t = up.tile([128, NB], F32, tag="u")
                        nc.sync.dma_start(out=u_t, in_=uR[:, cs : cs + NB])

                    # input layers + small heads first (only need x + small
                    # weights), so the e2w stream can still be in flight
                    h1 = hp.tile([128, 8, NB], F8, tag="h")
                    for j in range(4):
                        packed_pair(e1w[:, j * 128 : (j + 1) * 128], x_t, e1b, h1, j)

                    # aux head (A(x) params)
                    ha1 = abp.tile([128, 2, NB], F8, tag="ab8")
                    packed_pair(a1w, x_t, a1b, ha1, 0)
                    ha2 = abp.tile([128, 2, NB], F32R, tag="ab")
                    dr_layer(
                        lambda g, mi: w8("a2", mi * 128, (mi + 1) * 128),
                        1, a2b, ha1, ha2, 2,
                    )

                    # B(x) head
                    hb1 = abp.tile([128, 2, NB], F8, tag="ab8")
                    packed_pair(b1w, x_t, b1b, hb1, 0)
                    hb2 = abp.tile([128, 2, NB], F8, tag="ab8")
                    dr_layer(
                        lambda g, mi: w8("b2", mi * 128, (mi + 1) * 128),
                        1, b2b, hb1, hb2, 2,
                    )

                    # F | P | Q heads: psum [64, NB]; rows 0-31 drive F (via
                    # tanh), rows 32-63 drive both P and Q (phase-shifted sins)
                    pfpq = pmid.tile([2 * Z, NB], F32, tag="pm")
                    for k in range(2):
                        nc.tensor.matmul(
                            pfpq, fpqw[:, k, :], ha2[:, k, :],
                            start=(k == 0), stop=(k == 1),
                        )
                    # f = exp(a*DT) = (1+u)/(1-u), u = tanh(a*DT/2): keeps
                    # Silu/Sin/Tanh in one ACT table set (no table reloads)
                    tn_t = mp.tile([Z, NB], F32, tag="TN")
                    nc.scalar.activation(
                        tn_t, pfpq[:Z], AF.Tanh, bias=fb, scale=DT / 2
                    )
                    p_t = mp.tile([Z, NB], F32, tag="P")
                    nc.scalar.activation(p_t, pfpq[Z:], AF.Sin, bias=pb_hi, scale=DT)
                    q_t = mp.tile([Z, NB], F32, tag="Q")
                    nc.scalar.activation(q_t, pfpq[Z:], AF.Sin, bias=qb_hi, scale=DT)
                    dn_t = mp.tile([Z, NB], F32, tag="DN")
                    nc.vector.tensor_scalar(
                        out=dn_t[:], in0=tn_t[:], scalar1=-1.0, scalar2=1.0,
                        op0=ALU.mult, op1=ALU.add,
                    )
                    rd_t = mp.tile([Z, NB], F32, tag="RD")
                    nc.vector.reciprocal(out=rd_t[:], in_=dn_t[:])
                    f_t = mp.tile([Z, NB], F32, tag="F")
                    nc.vector.scalar_tensor_tensor(
                        out=f_t[:], in0=tn_t[:], scalar=1.0, in1=rd_t[:],
                        op0=ALU.add, op1=ALU.mult,
                    )

                    # big encoder layer (fp8 DoubleRow)
                    h2 = hp.tile([128, 8, NB], F8, tag="h")
                    dr_layer(
                        lambda g, mi: e2w[:, 2 * g : 2 * g + 2,
                                          mi * 128 : (mi + 1) * 128],
                        4, e2b, h1, h2, 8,
                    )

                    # z pair-broadcasts Z0|Z1 in one [64, NB] psum (fp8 DR)
                    pz = pmid.tile([2 * Z, NB], F32, tag="pm")
                    for g in range(4):
                        nc.tensor.matmul(
                            pz, w8("z01", g * 64, (g + 1) * 64),
                            h2[:, 2 * g : 2 * g + 2, :],
                            start=(g == 0), stop=(g == 3),
                            perf_mode=DR,
                        )
                    z0_t = mp.tile([Z, NB], F32, tag="Z0")
                    nc.vector.tensor_scalar(
                        out=z0_t[:], in0=pz[:Z], scalar1=INV8,
                        scalar2=z01b[:Z, 0:1], op0=ALU.mult, op1=ALU.add,
                    )
                    z1_t = mp.tile([Z, NB], F32, tag="Z1")
                    nc.vector.tensor_scalar(
                        out=z1_t[:], in0=pz[Z:], scalar1=INV8,
                        scalar2=z01b[Z:, 0:1], op0=ALU.mult, op1=ALU.add,
                    )

                    # Bflat (fp8 DR) + product tiles + Bu segment-sum (fp8 DR)
                    pr_t = prp.tile([128, 4, NB], F8, tag="prod")
                    for mc in range(4):
                        psb = pbig.tile([128, NB], F32, tag="pb")
                        nc.tensor.matmul(
                            psb, w8("b3", mc * 128, (mc + 1) * 128),
                            hb2[:, 0:2, :],
                            start=True, stop=True, perf_mode=DR,
                        )
                        nc.vector.scalar_tensor_tensor(
                            out=pr_t[:, mc, :], in0=psb[:],
                            scalar=b3b[:, mc : mc + 1],
                            in1=u_t[:], op0=ALU.add, op1=ALU.mult,
                        )
                    pbu = pmid.tile([Z, NB], F32, tag="pm")
                    for g in range(2):
                        nc.tensor.matmul(
                            pbu, w8("seg", g * 32, (g + 1) * 32),
                            pr_t[:, 2 * g : 2 * g + 2, :],
                            start=(g == 0), stop=(g == 1),
                            perf_mode=DR,
                        )

                    # z_next = G0*Z0 + G1*Z1 + (DT/PR_SCALE)*Bu
                    nc.vector.tensor_tensor(
                        out=p_t[:], in0=f_t[:], in1=p_t[:], op=ALU.mult
                    )
                    nc.vector.tensor_scalar(
                        out=p_t[:], in0=p_t[:], scalar1=DT, scalar2=m0b,
                        op0=ALU.mult, op1=ALU.add,
                    )
                    nc.vector.tensor_tensor(
                        out=q_t[:], in0=f_t[:], in1=q_t[:], op=ALU.mult
                    )
                    nc.vector.tensor_scalar(
                        out=q_t[:], in0=q_t[:], scalar1=DT, scalar2=m1b,
                        op0=ALU.mult, op1=ALU.add,
                    )
                    nc.vector.tensor_tensor(
                        out=p_t[:], in0=p_t[:], in1=z0_t[:], op=ALU.mult
                    )
                    nc.vector.tensor_tensor(
                        out=q_t[:], in0=q_t[:], in1=z1_t[:], op=ALU.mult
                    )
                    nc.vector.tensor_tensor(
                        out=p_t[:], in0=p_t[:], in1=q_t[:], op=ALU.add
                    )
                    zn_t = znp.tile([128, NB], F32R, tag="zn")
                    nc.vector.scalar_tensor_tensor(
                        out=zn_t[:Z], in0=pbu[:], scalar=DT / PR_SCALE,
                        in1=p_t[:], op0=ALU.mult, op1=ALU.add,
                    )
                    # replicate to all 4 row-groups for quad-packed d1
                    nc.vector.tensor_copy(out=zn_t[Z : 2 * Z], in_=zn_t[:Z])
                    nc.vector.tensor_copy(out=zn_t[2 * Z :], in_=zn_t[: 2 * Z])
                    zn_tiles.append(zn_t)

                # phase B: decoders, pipelined behind phase A on PE
                for c in range(NCHUNK):
                    cs = c * NB
                    zn_t = zn_tiles[c]
                    hd1 = hp.tile([128, 8, NB], F8, tag="h")
                    for g in range(2):
                        pss = [
                            pbig.tile([128, NB], F32, tag="pb", name=f"d1ps{_r}")
                            for _r in range(4)
                        ]
                        for r in range(4):
                            nc.tensor.matmul(
                                pss[r],
                                d1w[32 * r : 32 * r + 32, g * 128 : (g + 1) * 128],
                                zn_t[32 * r : 32 * r + 32, :],
                                start=True, stop=True,
                                tile_position=(32 * r, 0),
                            )
                        for r in range(4):
                            mi = 4 * g + r
                            nc.scalar.activation(
                                hd1[:, mi, :], pss[r], AF.Silu,
                                bias=d1b[:, mi : mi + 1], scale=1.0,
                            )
                    hd2 = hp.tile([128, 8, NB], F8, tag="h")
                    dr_layer(
                        lambda g, mi: d2w[:, 2 * g : 2 * g + 2,
                                          mi * 128 : (mi + 1) * 128],
                        4, d2b, hd1, hd2, 8,
                    )
                    hd3 = hp.tile([128, 8, NB], F32R, tag="h32")
                    dr_layer(
                        lambda g, mi: d3w[:, 2 * g : 2 * g + 2,
                                          mi * 128 : (mi + 1) * 128],
                        4, d3b, hd2, hd3, 8,
                    )

                    py_full = pbig.tile([128, NB], F32, tag="pb")
                    py_t = py_full[:X]
                    for k in range(8):
                        nc.tensor.matmul(
                            py_t, d4w[:, k, :], hd3[:, k, :],
                            start=(k == 0), stop=(k == 7),
                        )
                    y_sb = yp.tile([X, NB], F32, tag="y")
                    nc.vector.tensor_scalar_add(
                        out=y_sb[:], in0=py_t[:], scalar1=d4b
                    )
                    nc.sync.dma_start(out=yT[:, cs : cs + NB], in_=y_sb)

    nc.compile()
    return nc


def _prep_host(inputs):
    import ml_dtypes

    f32 = np.float32
    E4 = ml_dtypes.float8_e4m3
    FP8CLIP = 240.0

    x = np.asarray(inputs["x"], f32)
    u = np.asarray(inputs["u"], f32)

    xT = np.ascontiguousarray(x.T)
    x2T = np.concatenate([xT, xT], axis=0)  # [128, B]: x twice (row packing)
    # u tiled 8x along features; PR_SCALE/S8 folds the fp8 b3 descale and
    # ranges the fp8 product tiles
    uR = np.tile(np.ascontiguousarray(u.T) * (PR_SCALE / S8), (8, 1))

    def fm(w):
        """[K, M] -> [128, (K//128)*M]: per-partition-contiguous lhsT chunks."""
        kc = w.shape[0] // 128
        return np.ascontiguousarray(
            w.reshape(kc, 128, w.shape[1]).transpose(1, 0, 2).reshape(128, -1)
        )

    def fm3(w):
        """[K, M] -> [128, K//128, M]"""
        kc = w.shape[0] // 128
        return w.reshape(kc, 128, w.shape[1]).transpose(1, 0, 2)

    def q8(a):
        return np.asarray(
            np.clip(np.asarray(a, f32) * S8, -FP8CLIP, FP8CLIP), E4
        )

    def pack_pairs(w):
        """[64, M] -> [128, M//256, 128] row-packed pairs of 128-col chunks."""
        mt = w.shape[1] // 256
        out = np.zeros((128, mt, 128), f32)
        for j in range(mt):
            out[:64, j] = w[:, (2 * j) * 128 : (2 * j + 1) * 128]
            out[64:, j] = w[:, (2 * j + 1) * 128 : (2 * j + 2) * 128]
        return out

    idx0 = np.arange(Z) // 2 * 2
    idx1 = idx0 + 1
    even = (np.arange(Z) % 2 == 0).astype(f32)

    e_w3 = np.asarray(inputs["e_w3"], f32)
    e_b3 = np.asarray(inputs["e_b3"], f32)
    a_w3 = np.asarray(inputs["a_w3"], f32)
    a_b3 = np.asarray(inputs["a_b3"], f32)

    pi = np.pi

    wpack = np.zeros((128, WCOLS), f32)
    wpack[:, OFF["e1"] : OFF["e1"] + 512] = pack_pairs(
        np.asarray(inputs["e_w1"], f32)
    ).reshape(128, 512)
    wpack[:, OFF["a1"] : OFF["a1"] + 128] = pack_pairs(
        np.asarray(inputs["a_w1"], f32)
    )[:, 0]
    wpack[:, OFF["b1"] : OFF["b1"] + 128] = pack_pairs(
        np.asarray(inputs["b_w1"], f32)
    )[:, 0]
    wpack[:, OFF["fpq"] : OFF["fpq"] + 128] = fm(
        np.concatenate([a_w3[:, idx0], a_w3[:, idx1]], axis=1)
    )
    d_w1 = np.asarray(inputs["d_w1"], f32)
    for g in range(2):
        for r in range(4):
            m = 4 * g + r
            wpack[
                32 * r : 32 * r + 32,
                OFF["d1"] + g * 128 : OFF["d1"] + (g + 1) * 128,
            ] = d_w1[:, m * 128 : (m + 1) * 128]
    wpack[:, OFF["d4"] : OFF["d4"] + 512] = fm(np.asarray(inputs["d_w4"], f32))

    # fp8 pack: [128, 2, W8COLS], dim1 = k-group within a DoubleRow pair
    wp8 = np.zeros((128, 2, W8COLS), f32)
    wp8[:, :, OFF8["a2"] : OFF8["a2"] + 256] = fm3(
        np.asarray(inputs["a_w2"], f32) * S8
    )
    wp8[:, :, OFF8["b2"] : OFF8["b2"] + 256] = fm3(
        np.asarray(inputs["b_w2"], f32) * S8
    )
    wp8[:, :, OFF8["b3"] : OFF8["b3"] + 512] = fm3(
        np.asarray(inputs["b_w3"], f32) * S8
    )
    # segment-sum 0/1 weights (unscaled): pair g, group i -> product chunk
    # mc = 2g+i; within-chunk output row m = 8*mc + p//16
    for g2 in range(2):
        for i in range(2):
            mc = 2 * g2 + i
            for p in range(128):
                wp8[p, i, OFF8["seg"] + g2 * 32 + 8 * mc + p // 16] = 1.0
    e3cat = np.concatenate([e_w3[:, idx0], e_w3[:, idx1]], axis=1)  # [1024, 64]
    e3v = e3cat.reshape(8, 128, 64) * S8
    for g2 in range(4):
        for i in range(2):
            wp8[:, i, OFF8["z01"] + g2 * 64 : OFF8["z01"] + (g2 + 1) * 64] = e3v[
                2 * g2 + i
            ]
    wpack8 = np.asarray(
        np.clip(wp8, -FP8CLIP, FP8CLIP), E4
    ).reshape(128, 2 * W8COLS)

    def bcol(b):
        return np.asarray(b, f32).reshape(-1, 128).T

    bpack = np.zeros((128, BCOLS), f32)
    bpack[:, 0:8] = bcol(inputs["e_b1"])
    bpack[:, 8:16] = bcol(inputs["e_b2"])
    bpack[:, 16:18] = bcol(inputs["a_b1"])
    bpack[:, 18:20] = bcol(inputs["a_b2"])
    bpack[:, 20:22] = bcol(inputs["b_b1"])
    bpack[:, 22:24] = bcol(inputs["b_b2"])
    bpack[:, 24:28] = bcol(inputs["b_b3"]) * S8  # folded with psb = S8*pre
    bpack[:, 28:36] = bcol(inputs["d_b1"])
    bpack[:, 36:44] = bcol(inputs["d_b2"])
    bpack[:, 44:52] = bcol(inputs["d_b3"])
    bpack[:64, 52] = np.concatenate([e_b3[idx0], e_b3[idx1]])
    bpack[:32, 53] = DT * a_b3[idx0] / 2  # tanh half-angle bias
    bpack[32:64, 54] = DT * a_b3[idx1] + even * (pi / 2)
    bpack[32:64, 55] = DT * a_b3[idx1] + np.where(even, pi, pi / 2)
    bpack[:32, 56] = even
    bpack[:32, 57] = 1.0 - even
    bpack[:64, 58] = np.asarray(inputs["d_b4"], f32)

    shared = {
        "wpack": wpack,
        "wpack8": wpack8,
        "bpack": bpack,
        "w_e2": q8(fm(np.asarray(inputs["e_w2"], f32))),
        "w_d2": q8(fm(np.asarray(inputs["d_w2"], f32))),
        "w_d3": q8(fm(np.asarray(inputs["d_w3"], f32))),
    }

    in_maps = []
    for c in range(N_CORES):
        sl = slice(c * BC, (c + 1) * BC)
        m = dict(shared)
        m["x2T"] = np.ascontiguousarray(x2T[:, sl])
        m["uR"] = np.ascontiguousarray(uR[:, sl])
        in_maps.append(m)
    return in_maps


def kernel(**inputs) -> np.ndarray:
    from concourse import bass_utils

    if "nc" not in _CACHE:
        _CACHE["nc"] = _build()
    nc = _CACHE["nc"]
    in_maps = _prep_host(inputs)
    res = bass_utils.run_bass_kernel_spmd(
        nc, in_maps, core_ids=list(range(N_CORES))
    )
    return np.concatenate(
        [np.asarray(res.results[c]["yT"]).T for c in range(N_CORES)], axis=0
    ).astype(np.float32)
